# revision 38
# baseline (speedup 1.0000x reference)
"""Trainium2 Bass kernel for gated attention (dense_transformer).

Module: LayerNorm -> fused QKV -> per-head scaled-dot-product attention with
additive bias + key mask -> sigmoid(gate) * attn_out -> output projection.

Shapes (hardcoded): B=1, N=2048, D=1024, H=16, HW=64.

Sharding: 2 heads per core across 8 cores (tensor-parallel over H).  Each core
computes a partial o_proj contribution over its 128 local channels; the host
sums the 8 partials and adds b_o.

Device-side math per core c (heads h0=2c, h1=2c+1):
  - x arrives transposed (D, N): channels on partitions; fp32 bits tagged
    float32r so the PE runs matmuls at 1 cycle/row (bf16 rate).
  - LN stats via ones-matmuls on PE: s1=sum(x), s2=sum(x^2) per token.
    rstd = rsqrt((s2 - s1^2/D)/D + eps).  Mean subtraction is folded into the
    QKV/gate matmuls as a rank-1 PSUM-accumulated correction
    (-colsum(W)/D) (x) s1; ln_w is folded into W on the host; the 1/sqrt(HW)
    q-scale is folded into W's q-rows on the host.
  - QKV^T = (W_c x)^T accumulated in PSUM, evicted with *rstd broadcast.
  - Scores computed TRANSPOSED: S^T[k,q] = sum_c kT[c,k] qT[c,q] (PE),
    P^T = exp(S^T) (ACT, PSUM->SBUF bf16) * expb (DVE bf16 2x), where
    expb[k,q] = exp(bias[h,q,k]) * mask[k] is precomputed on host in bf16
    (masked keys become exact zeros; softmax needs no max pass at these
    logit scales).
  - AV: y_un^T = [v | 1]-augmented matmul accumulating over key chunks;
    row 64 of the PSUM result is the softmax denominator.
  - gate^T = sigmoid(rstd*(W_g x - rank1) + b_g-per-partition) (ACT).
  - gy^T = gT * (1/den broadcast via tiny select matmul) * yT (DVE).
  - out_partial = gy^T.T @ WoT via PE, evicted ACT/DVE, DMA'd out.
"""

import numpy as np
import ml_dtypes

B, N, D, H, HW = 1, 2048, 1024, 16, 64
EPS = 1e-5
NCORES = 8
HPC = H // NCORES          # heads per core = 2
QB = 512                   # q free-dim block (one PSUM bank of fp32)
NQ = N // QB               # 4
CPD = D // 128             # 8 channel chunks
NT = N // 128              # 16 token tiles

_CACHE = {}


def _host_prep(x, bias, mask, ln_w, ln_b, W_qkv, W_o, b_o, W_g, b_g):
    """Build per-core input maps. Returns (in_maps, KC, has_cb)."""
    f32 = np.float32
    bf16 = ml_dtypes.bfloat16
    x = np.asarray(x, f32)
    bias = np.asarray(bias, f32)
    maskv = np.asarray(mask).reshape(B, N)[0].astype(np.int64)
    ln_w = np.asarray(ln_w, f32)
    ln_b = np.asarray(ln_b, f32)
    W_qkv = np.asarray(W_qkv, f32)
    W_g = np.asarray(W_g, f32)
    W_o = np.asarray(W_o, f32)

    valid = np.nonzero(maskv != 0)[0]
    L = int(valid[-1]) + 1 if valid.size else 128
    KC = (L + 127) // 128
    Lp = KC * 128

    xT = np.ascontiguousarray(x[0].T)                      # (D, N)

    Wl = W_qkv * ln_w[None, :]                             # (3D, D)
    has_cb = bool(np.any(ln_b != 0.0))

    # expb for all heads: (H, Lp, N) bf16 = exp(bias[h, q, k]).T * mask[k]
    mk = (maskv[:L] != 0).astype(f32)
    expb_all = np.zeros((H, Lp, N), dtype=bf16)
    for h in range(H):
        eb = np.exp(bias[0, h, :, :L].T.astype(f32)) * mk[:, None]
        expb_all[h, :L, :] = eb.astype(bf16)

    in_maps = []
    for c in range(NCORES):
        h0, h1 = HPC * c, HPC * c + 1
        rows = []
        scale = []
        for part, s in ((128, 1.0), (0, HW ** -0.5), (64, 1.0)):
            # order: [v_h0 v_h1 | q_h0 q_h1 | k_h0 k_h1]
            for h in (h0, h1):
                rows.append(np.arange(h * 192 + part, h * 192 + part + 64))
                scale.append(np.full(64, s, f32))
        rows = np.concatenate(rows)
        scale = np.concatenate(scale)
        Wc = Wl[rows] * scale[:, None]                      # (384, D)
        wqkvT = np.ascontiguousarray(Wc.T)                  # (D, 384)
        csqkv = -Wc.sum(axis=1) / D                         # (384,)
        cbqkv = (W_qkv[rows] @ ln_b) * scale                # (384,)

        gsl = slice(c * 128, (c + 1) * 128)
        Wgc = (W_g * ln_w[None, :])[gsl]                    # (128, D)
        wgT = np.ascontiguousarray(Wgc.T)                   # (D, 128)
        csg = -Wgc.sum(axis=1) / D                          # (128,)
        cgb = ((W_g[gsl] @ ln_b + np.asarray(b_g, f32)[gsl]) / 2.0).reshape(128, 1)

        woT = np.ascontiguousarray(W_o[:, gsl].T)           # (128, D)

        # cvec (65, 1152): r0=[csqkv|csg|-|cbqkv], r32=[sel_a|sel_b], r64=onesr.
        cvec = np.zeros((65, 1152), f32)
        cvec[0, 0:384] = csqkv
        cvec[0, 384:512] = csg
        cvec[32, 512:576] = 1.0         # sel_a: gy rows 0:64  <- 1/den_h0
        cvec[32, 704:768] = 1.0         # sel_b: gy rows 64:128 <- 1/den_h1
        cvec[0, 768:1152] = cbqkv
        cvec[64, 0:128] = 1.0           # onesr (pairs rstd @ p64)
        # cmat (128, 129): [:,0:128]=identity, col 128 = ones column
        cmat = np.zeros((128, 129), f32)
        cmat[:, 0:128] = np.eye(128, dtype=f32)
        cmat[:, 128] = 1.0
        m = {
            "xT": xT,
            "wqkvT": wqkvT,
            "wgT": wgT,
            "woT": woT,
            "expb": np.ascontiguousarray(expb_all[h0:h1 + 1]),
            "cvec": cvec,
            "cmat": cmat,
            "cgb": np.ascontiguousarray(cgb),
        }
        in_maps.append(m)
    return in_maps, KC, has_cb


def _build(KC, has_cb):
    import concourse.bass as bass
    import concourse.mybir as mybir
    import concourse.tile as tile
    from concourse import bacc

    f32 = mybir.dt.float32
    f32r = mybir.dt.float32r
    bf16 = mybir.dt.bfloat16
    AF = mybir.ActivationFunctionType
    ALU = mybir.AluOpType

    nc = bacc.Bacc("TRN2", target_bir_lowering=False)

    xT_d = nc.declare_dram_parameter("xT", [D, N], f32r, False)
    wqkvT_d = nc.declare_dram_parameter("wqkvT", [D, 384], f32r, False)
    wgT_d = nc.declare_dram_parameter("wgT", [D, 128], f32r, False)
    woT_d = nc.declare_dram_parameter("woT", [128, D], f32r, False)
    expb_d = nc.declare_dram_parameter("expb", [HPC, KC * 128, N], bf16, False)
    cvec_d = nc.declare_dram_parameter("cvec", [65, 1152], f32r, False)
    cmat_d = nc.declare_dram_parameter("cmat", [128, 129], f32r, False)
    cgb_d = nc.declare_dram_parameter("cgb", [128, 1], f32, False)
    out_d = nc.declare_dram_parameter("out", [N, D], bf16, True)

    with tile.TileContext(nc) as tc:
        with (
            nc.allow_low_precision(reason="f32r rounding feeds PE-rate matmuls"),
            tc.tile_pool(name="big", bufs=1) as big,
            tc.tile_pool(name="small", bufs=1) as small,
            tc.tile_pool(name="pT", bufs=3) as pTp,
            tc.tile_pool(name="expb", bufs=3) as ebp,
            tc.tile_pool(name="outs", bufs=3) as outs,
            tc.tile_pool(name="work", bufs=2) as work,
            tc.tile_pool(name="xsqp", bufs=4) as xsqp,
        ):
            # ---------------- persistent SBUF ----------------
            xT = big.tile([128, CPD * N], f32r, tag="xT")      # 64K/part
            qT = big.tile([128, N], bf16, tag="qT")
            kT = big.tile([128, N], bf16, tag="kT")
            vT = big.tile([128, N], f32r, tag="vT")
            gT = big.tile([128, N], f32r, tag="gT")
            yT = big.tile([128, N], f32, tag="yT")
            rstdb = big.tile([128, N], f32, tag="rstdb")
            vaug = big.tile([128, HPC * KC * 65], bf16, tag="vaug")
            wqkvT = big.tile([128, CPD * 384], f32r, tag="wqkvT")
            wgT = big.tile([128, CPD * 128], f32r, tag="wgT")
            woT = small.tile([128, D], f32r, tag="woT")
            cvec = small.tile([65, 1152], f32r, tag="cvec")
            cmat = small.tile([128, 129], f32r, tag="cmat")
            cgb = small.tile([128, 1], f32, tag="cgb")
            zcol = small.tile([128, 1], f32, tag="zcol")
            nc.vector.memset(zcol, 0.0)
            # stats slab (65, 3N) f32r: column slabs at bases 0/32/64 chosen
            # so every multi-input DVE op sees equal input bases.
            stats = small.tile([65, 3 * N], f32r, tag="stats")
            epsc = small.tile([65, 1], f32, tag="epsc")
            nc.vector.memset(epsc, EPS)
            csqkv = cvec[0:1, 0:384]
            csg = cvec[0:1, 384:512]
            sel_a = cvec[32:33, 512:640]
            sel_b = cvec[32:33, 640:768]
            cbq = cvec[0:1, 768:1152]
            onesr = cvec[64:65, 0:128]
            ident = cmat[:, 0:128]
            onesc = cmat[:, 128:129]
            s1 = stats[0:1, 0:N]                  # @p0
            s2 = stats[32:33, 0:N]                # @p32
            t1 = stats[32:33, N:2 * N]            # @p32
            t2 = stats[64:65, 0:N]                # @p64; becomes rstd in place
            rstd = t2
            sqv = stats[0:1, N:2 * N]             # @p0 (pairs cbq)
            dens = stats[32:33, 0:2 * N]          # aliases s2/t1 (dead) @p32
            dpt = stats[32:33, 2 * N:2 * N + 2 * QB]    # @p32 (pairs sels)

            # ---------------- load DMAs (dependency-first order) ----------------
            nc.sync.dma_start(out=cmat, in_=cmat_d.ap())
            nc.sync.dma_start(out=cvec, in_=cvec_d.ap())
            nc.sync.dma_start(out=cgb, in_=cgb_d.ap())
            xTv = xT_d.ap().rearrange("(c p) q -> c p q", p=128)
            wv = wqkvT_d.ap().rearrange("(c p) m -> c p m", p=128)
            gv = wgT_d.ap().rearrange("(c p) m -> c p m", p=128)
            for i in range(CPD):
                nc.sync.dma_start(out=xT[:, i * N:(i + 1) * N], in_=xTv[i])
                nc.sync.dma_start(out=wqkvT[:, i * 384:(i + 1) * 384], in_=wv[i])
                nc.sync.dma_start(out=wgT[:, i * 128:(i + 1) * 128], in_=gv[i])
            nc.sync.dma_start(out=woT, in_=woT_d.ap())

            # ---------------- phase A: stats ----------------
            with tc.tile_pool(name="psA", bufs=4, space="PSUM") as psA, \
                 tc.high_priority():
                # per-qc chains so rstd[qc] unblocks as soon as its stats land
                for qc in range(NQ):
                    q0, q1 = qc * QB, (qc + 1) * QB
                    ps = psA.tile([1, QB], f32, tag="stat")
                    for i in range(CPD):
                        nc.tensor.matmul(
                            ps, onesc, xT[:, i * N + q0:i * N + q1],
                            start=(i == 0), stop=(i == CPD - 1))
                    nc.scalar.copy(out=s1[:, q0:q1], in_=ps)
                    ps2 = psA.tile([1, QB], f32, tag="stat")
                    for i in range(CPD):
                        xs = xsqp.tile([128, QB], f32r, tag="xsq")
                        sl = xT[:, i * N + q0:i * N + q1]
                        if i % 2 == 0:
                            nc.vector.tensor_mul(xs, sl, sl)
                        else:
                            nc.scalar.square(xs, sl)
                        nc.tensor.matmul(ps2, onesc, xs,
                                         start=(i == 0), stop=(i == CPD - 1))
                    nc.scalar.copy(out=s2[:, q0:q1], in_=ps2)
                # one wide op per chain stage (avoids per-qc ping-pong)
                nc.vector.tensor_mul(t1, s1, s1)
                nc.vector.scalar_tensor_tensor(
                    t2, t1, -1.0 / D, s2, op0=ALU.mult, op1=ALU.add)
                if has_cb:
                    tv = work.tile([1, N], f32r, tag="tv")
                    nc.vector.tensor_scalar(
                        tv, t2, 1.0 / D, EPS, op0=ALU.mult, op1=ALU.add)
                # rstd = 1/sqrt(var+eps) in one ACT op (one table set)
                nc.scalar.activation(rstd, t2, AF.Abs_reciprocal_sqrt,
                                     bias=epsc[64:65], scale=1.0 / D)
                if has_cb:
                    nc.vector.tensor_mul(sqv, tv, rstd)
                for qc in range(NQ):
                    q0, q1 = qc * QB, (qc + 1) * QB
                    pb = psA.tile([128, QB], f32, tag="rb")
                    nc.tensor.matmul(pb, onesr, rstd[:, q0:q1],
                                     start=True, stop=True)
                    nc.vector.tensor_copy(rstdb[:, q0:q1], pb)

            # ---------------- phase B: QKV + gate ----------------
            with tc.tile_pool(name="psB", bufs=3, space="PSUM") as psB:
                def vtrans_now():
                    for h in range(HPC):
                        for kc in range(KC):
                            pv = psB.tile([128, 64], f32r, tag="vtr")
                            nc.tensor.transpose(
                                pv,
                                vT[h * 64:(h + 1) * 64,
                                   kc * 128:(kc + 1) * 128],
                                ident[h * 64:(h + 1) * 64,
                                      h * 64:(h + 1) * 64])
                            base = (h * KC + kc) * 65
                            nc.vector.tensor_copy(vaug[:, base:base + 64],
                                                  pv.bitcast(f32))
                            nc.vector.memset(vaug[:, base + 64:base + 65], 1.0)
                dests = (vT, qT, kT)
                for j in range(3):          # v | q | k column blocks (128 each)
                    for qc in range(NQ):
                        ps = psB.tile([128, QB], f32, tag="qkv")
                        for i in range(CPD):
                            nc.tensor.matmul(
                                ps,
                                wqkvT[:, i * 384 + j * 128:i * 384 + (j + 1) * 128],
                                xT[:, i * N + qc * QB:i * N + (qc + 1) * QB],
                                start=(i == 0), stop=False)
                        nc.tensor.matmul(
                            ps, csqkv[:, j * 128:(j + 1) * 128],
                            s1[:, qc * QB:(qc + 1) * QB],
                            start=False, stop=(not has_cb))
                        if has_cb:
                            nc.tensor.matmul(
                                ps, cbq[:, j * 128:(j + 1) * 128],
                                sqv[:, qc * QB:(qc + 1) * QB],
                                start=False, stop=True)
                        nc.vector.tensor_mul(
                            dests[j][:, qc * QB:(qc + 1) * QB], ps,
                            rstdb[:, qc * QB:(qc + 1) * QB])
                    if j == 0:
                        vtrans_now()
                # (vtrans emitted inside j==0 via vtrans_now)
                for h in []:
                    for kc in range(KC):
                        pv = psB.tile([128, 64], f32r, tag="vtr")
                        nc.tensor.transpose(
                            pv,
                            vT[h * 64:(h + 1) * 64, kc * 128:(kc + 1) * 128],
                            ident[h * 64:(h + 1) * 64, h * 64:(h + 1) * 64])
                        base = (h * KC + kc) * 65
                        nc.vector.tensor_copy(vaug[:, base:base + 64],
                                              pv.bitcast(f32))
                        nc.vector.memset(vaug[:, base + 64:base + 65], 1.0)

            # ---------------- phase C: attention ----------------
            def emit_gate():
                # gate matmuls act as PE filler while ACT grinds exp
                with tc.tile_pool(name="psG", bufs=2, space="PSUM") as psG:
                    for qc in range(NQ):
                        ps = psG.tile([128, QB], f32, tag="gate")
                        for i in range(CPD):
                            nc.tensor.matmul(
                                ps, wgT[:, i * 128:(i + 1) * 128],
                                xT[:, i * N + qc * QB:i * N + (qc + 1) * QB],
                                start=(i == 0), stop=False)
                        nc.tensor.matmul(
                            ps, csg, s1[:, qc * QB:(qc + 1) * QB],
                            start=False, stop=True)
                        gs = gT[:, qc * QB:(qc + 1) * QB]
                        nc.vector.tensor_mul(
                            gs, ps, rstdb[:, qc * QB:(qc + 1) * QB])
                        # sigmoid(z) = 0.5 + 0.5*tanh(z/2); tanh shares the
                        # exp table set so ACT never switches sets mid-kernel
                        nc.scalar.activation(gs, gs, AF.Tanh, bias=cgb,
                                             scale=0.5)
                        nc.vector.tensor_scalar(
                            gs, gs, 0.5, 0.5,
                            op0=ALU.mult, op1=ALU.add)

            for h in range(HPC):
                with tc.tile_pool(name=f"psC{h}", bufs=2, space="PSUM") as psC, \
                     tc.tile_pool(name=f"psAV{h}", bufs=1, space="PSUM") as psAV:
                    avs = []
                    for _qc in range(NQ):
                        av_t = psAV.tile([65, QB], f32, tag=f"av{_qc}")
                        avs.append(av_t)
                    for kc in range(KC):
                        eb = ebp.tile([128, N], bf16, tag="expb")
                        nc.sync.dma_start(
                            out=eb, in_=expb_d.ap()[h, kc * 128:(kc + 1) * 128, :])
                        pT = pTp.tile([128, N], bf16, tag="pT")
                        for half in range(2):
                            sps = psC.tile([128, 2 * QB], f32, tag="sT")
                            for qq in range(2):
                                qc = half * 2 + qq
                                nc.tensor.matmul(
                                    sps[:, qq * QB:(qq + 1) * QB],
                                    kT[h * 64:(h + 1) * 64, kc * 128:(kc + 1) * 128],
                                    qT[h * 64:(h + 1) * 64, qc * QB:(qc + 1) * QB],
                                    start=True, stop=True)
                            pe = pT[:, half * 2 * QB:(half + 1) * 2 * QB]
                            nc.scalar.activation(
                                pe, sps, AF.Exp, bias=zcol, scale=1.0)
                            nc.vector.tensor_mul(
                                pe, pe, eb[:, half * 2 * QB:(half + 1) * 2 * QB])
                        vbase = (h * KC + kc) * 65
                        for qc in range(NQ):
                            nc.tensor.matmul(
                                avs[qc], vaug[:, vbase:vbase + 65],
                                pT[:, qc * QB:(qc + 1) * QB],
                                start=(kc == 0), stop=(kc == KC - 1))
                    for qc in range(NQ):
                        nc.vector.tensor_copy(
                            yT[h * 64:(h + 1) * 64, qc * QB:(qc + 1) * QB],
                            avs[qc][0:64, :])
                        nc.vector.tensor_copy(
                            dens[:, h * N + qc * QB:h * N + (qc + 1) * QB],
                            avs[qc][64:65, :])
                if h == 0:
                    emit_gate()

            # ---------------- phase D: gy + out projection ----------------
            with tc.tile_pool(name="psD", bufs=2, space="PSUM") as psD, \
                 tc.tile_pool(name="psDo", bufs=2, space="PSUM") as psDo:
                for qc in range(NQ):
                    dp0 = dpt[:, 0 * QB:1 * QB]
                    dp1 = dpt[:, 1 * QB:2 * QB]
                    nc.vector.reciprocal(
                        dp0, dens[:, 0 * N + qc * QB:0 * N + (qc + 1) * QB])
                    nc.vector.reciprocal(
                        dp1, dens[:, 1 * N + qc * QB:1 * N + (qc + 1) * QB])
                    pi = psD.tile([128, QB], f32, tag="invb")
                    nc.tensor.matmul(pi, sel_a, dp0, start=True, stop=False)
                    nc.tensor.matmul(pi, sel_b, dp1, start=False, stop=True)
                    gs = gT[:, qc * QB:(qc + 1) * QB]
                    nc.vector.tensor_mul(gs, gs, pi.bitcast(f32r))
                    nc.vector.tensor_mul(gs, gs, yT[:, qc * QB:(qc + 1) * QB]
                                         .bitcast(f32r))
                    # out partial for this qc's 4 token tiles immediately
                    for tt in range(qc * 4, qc * 4 + 4):
                        po = psDo.tile([128, D], f32, tag="outp")
                        for oc in range(2):
                            nc.tensor.matmul(
                                po[:, oc * QB:(oc + 1) * QB],
                                gT[:, tt * 128:(tt + 1) * 128],
                                woT[:, oc * QB:(oc + 1) * QB],
                                start=True, stop=True)
                        ot = outs.tile([128, D], bf16, tag="outsb")
                        if tt % 2 == 0:
                            nc.scalar.copy(out=ot, in_=po)
                        else:
                            nc.vector.tensor_copy(ot, po)
                        nc.sync.dma_start(
                            out=out_d.ap()[tt * 128:(tt + 1) * 128, :], in_=ot)

    nc.finalize()
    return nc


def _get_nc(KC, has_cb):
    key = (KC, has_cb)
    if key not in _CACHE:
        _CACHE[key] = _build(KC, has_cb)
    return _CACHE[key]


def _run(inputs, trace=False):
    from concourse.bass_utils import run_bass_kernel_spmd

    in_maps, KC, has_cb = _host_prep(**inputs)
    nc = _get_nc(KC, has_cb)
    res = run_bass_kernel_spmd(
        nc, in_maps, core_ids=list(range(NCORES)), trace=trace)
    acc = np.zeros((N, D), np.float64)
    for i in range(NCORES):
        acc += np.asarray(res.results[i]["out"], np.float64)
    out = acc.astype(np.float32) + np.asarray(inputs["b_o"], np.float32)[None, :]
    return out.reshape(B, N, D), res


def kernel(**inputs):
    out, _ = _run(inputs, trace=False)
    return out


def kernel_traced(**inputs):
    return _run(inputs, trace=True)


# revision 48
# speedup vs baseline: 136.2032x; 136.2032x over previous
"""Trainium2 Bass kernel for gated attention (dense_transformer).

Module: LayerNorm -> fused QKV -> per-head scaled-dot-product attention with
additive bias + key mask -> sigmoid(gate) * attn_out -> output projection.

Shapes (hardcoded): B=1, N=2048, D=1024, H=16, HW=64.

Sharding: 2 heads per core across 8 cores (tensor-parallel over H).  Each core
computes a partial o_proj contribution over its 128 local channels; the host
sums the 8 partials and adds b_o.

Device-side math per core c (heads h0=2c, h1=2c+1):
  - x arrives transposed (D, N): channels on partitions; fp32 bits tagged
    float32r so the PE runs matmuls at 1 cycle/row (bf16 rate).
  - LN stats via ones-matmuls on PE: s1=sum(x), s2=sum(x^2) per token.
    rstd = rsqrt((s2 - s1^2/D)/D + eps).  Mean subtraction is folded into the
    QKV/gate matmuls as a rank-1 PSUM-accumulated correction
    (-colsum(W)/D) (x) s1; ln_w is folded into W on the host; the 1/sqrt(HW)
    q-scale is folded into W's q-rows on the host.
  - QKV^T = (W_c x)^T accumulated in PSUM, evicted with *rstd broadcast.
  - Scores computed TRANSPOSED: S^T[k,q] = sum_c kT[c,k] qT[c,q] (PE),
    P^T = exp(S^T) (ACT, PSUM->SBUF bf16) * expb (DVE bf16 2x), where
    expb[k,q] = exp(bias[h,q,k]) * mask[k] is precomputed on host in bf16
    (masked keys become exact zeros; softmax needs no max pass at these
    logit scales).
  - AV: y_un^T = [v | 1]-augmented matmul accumulating over key chunks;
    row 64 of the PSUM result is the softmax denominator.
  - gate^T = sigmoid(rstd*(W_g x - rank1) + b_g-per-partition) (ACT).
  - gy^T = gT * (1/den broadcast via tiny select matmul) * yT (DVE).
  - out_partial = gy^T.T @ WoT via PE, evicted ACT/DVE, DMA'd out.
"""

import numpy as np
import ml_dtypes

B, N, D, H, HW = 1, 2048, 1024, 16, 64
EPS = 1e-5
NCORES = 8
HPC = H // NCORES          # heads per core = 2
QB = 512                   # q free-dim block (one PSUM bank of fp32)
NQ = N // QB               # 4
CPD = D // 128             # 8 channel chunks
NT = N // 128              # 16 token tiles

_CACHE = {}


def _host_prep(x, bias, mask, ln_w, ln_b, W_qkv, W_o, b_o, W_g, b_g):
    """Build per-core input maps. Returns (in_maps, KC, has_cb)."""
    f32 = np.float32
    bf16 = ml_dtypes.bfloat16
    x = np.asarray(x, f32)
    bias = np.asarray(bias, f32)
    maskv = np.asarray(mask).reshape(B, N)[0].astype(np.int64)
    ln_w = np.asarray(ln_w, f32)
    ln_b = np.asarray(ln_b, f32)
    W_qkv = np.asarray(W_qkv, f32)
    W_g = np.asarray(W_g, f32)
    W_o = np.asarray(W_o, f32)

    valid = np.nonzero(maskv != 0)[0]
    L = int(valid[-1]) + 1 if valid.size else 128
    KC = (L + 127) // 128
    Lp = KC * 128

    xT = np.ascontiguousarray(x[0].T.astype(bf16))         # (D, N) bf16

    Wl = W_qkv * ln_w[None, :]                             # (3D, D)
    has_cb = bool(np.any(ln_b != 0.0))

    # expb for all heads: (H, Lp, N) bf16 = exp(bias[h, q, k]).T * mask[k]
    mk = (maskv[:L] != 0).astype(f32)
    expb_all = np.zeros((H, Lp, N), dtype=bf16)
    for h in range(H):
        eb = np.exp(bias[0, h, :, :L].T.astype(f32)) * mk[:, None]
        expb_all[h, :L, :] = eb.astype(bf16)

    in_maps = []
    for c in range(NCORES):
        h0, h1 = HPC * c, HPC * c + 1
        rows = []
        scale = []
        for part, s in ((128, 1.0), (0, HW ** -0.5), (64, 1.0)):
            # order: [v_h0 v_h1 | q_h0 q_h1 | k_h0 k_h1]
            for h in (h0, h1):
                rows.append(np.arange(h * 192 + part, h * 192 + part + 64))
                scale.append(np.full(64, s, f32))
        rows = np.concatenate(rows)
        scale = np.concatenate(scale)
        Wc = Wl[rows] * scale[:, None]                      # (384, D)
        wqkvT = np.ascontiguousarray(Wc.T.astype(bf16))     # (D, 384)
        csqkv = -Wc.sum(axis=1) / D                         # (384,)
        cbqkv = (W_qkv[rows] @ ln_b) * scale                # (384,)

        gsl = slice(c * 128, (c + 1) * 128)
        Wgc = (W_g * ln_w[None, :])[gsl]                    # (128, D)
        wgT = np.ascontiguousarray(Wgc.T.astype(bf16))      # (D, 128)
        csg = -Wgc.sum(axis=1) / D                          # (128,)
        cgb = ((W_g[gsl] @ ln_b + np.asarray(b_g, f32)[gsl]) / 2.0).reshape(128, 1)

        woT = np.ascontiguousarray(W_o[:, gsl].T)           # (128, D)

        # cvec (65, 1152): r0=[csqkv|csg|-|cbqkv], r32=[sel_a|sel_b], r64=onesr.
        cvec = np.zeros((65, 1152), f32)
        cvec[0, 0:384] = csqkv
        cvec[0, 384:512] = csg
        cvec[32, 512:576] = 1.0         # sel_a: gy rows 0:64  <- 1/den_h0
        cvec[32, 704:768] = 1.0         # sel_b: gy rows 64:128 <- 1/den_h1
        cvec[0, 768:1152] = cbqkv
        cvec[64, 0:128] = 1.0           # onesr (pairs rstd @ p64)
        # cmat (128, 129): [:,0:128]=identity, col 128 = ones column
        cmat = np.zeros((128, 129), f32)
        cmat[:, 0:128] = np.eye(128, dtype=f32)
        cmat[:, 128] = 1.0
        m = {
            "xT": xT,
            "wqkvT": wqkvT,
            "wgT": wgT,
            "woT": woT,
            "expb": np.ascontiguousarray(expb_all[h0:h1 + 1]),
            "cvec": cvec,
            "cmat": cmat,
            "cgb": np.ascontiguousarray(cgb),
        }
        in_maps.append(m)
    return in_maps, KC, has_cb


def _build(KC, has_cb):
    import concourse.bass as bass
    import concourse.mybir as mybir
    import concourse.tile as tile
    from concourse import bacc

    f32 = mybir.dt.float32
    f32r = mybir.dt.float32r
    bf16 = mybir.dt.bfloat16
    AF = mybir.ActivationFunctionType
    ALU = mybir.AluOpType

    nc = bacc.Bacc("TRN2", target_bir_lowering=False)

    xT_d = nc.declare_dram_parameter("xT", [D, N], bf16, False)
    wqkvT_d = nc.declare_dram_parameter("wqkvT", [D, 384], bf16, False)
    wgT_d = nc.declare_dram_parameter("wgT", [D, 128], bf16, False)
    woT_d = nc.declare_dram_parameter("woT", [128, D], f32r, False)
    expb_d = nc.declare_dram_parameter("expb", [HPC, KC * 128, N], bf16, False)
    cvec_d = nc.declare_dram_parameter("cvec", [65, 1152], f32r, False)
    cmat_d = nc.declare_dram_parameter("cmat", [128, 129], f32r, False)
    cgb_d = nc.declare_dram_parameter("cgb", [128, 1], f32, False)
    out_d = nc.declare_dram_parameter("out", [N, D], bf16, True)

    with tile.TileContext(nc) as tc:
        with (
            nc.allow_low_precision(reason="f32r rounding feeds PE-rate matmuls"),
            tc.tile_pool(name="big", bufs=1) as big,
            tc.tile_pool(name="small", bufs=1) as small,
            tc.tile_pool(name="pT", bufs=3) as pTp,
            tc.tile_pool(name="expb", bufs=4) as ebp,
            tc.tile_pool(name="outs", bufs=4) as outs,
            tc.tile_pool(name="work", bufs=2) as work,
            tc.tile_pool(name="xsqp", bufs=4) as xsqp,
        ):
            # ---------------- persistent SBUF ----------------
            xT = big.tile([128, CPD * N], bf16, tag="xT")      # 32K/part
            qT = big.tile([128, N], bf16, tag="qT")
            kT = big.tile([128, N], bf16, tag="kT")
            vT = big.tile([128, N], f32r, tag="vT")
            gT = big.tile([128, N], f32r, tag="gT")
            yT = big.tile([128, N], f32, tag="yT")
            rstdb = big.tile([128, N], f32, tag="rstdb")
            vaug = big.tile([128, HPC * KC * 65], bf16, tag="vaug")
            wqkvT = big.tile([128, CPD * 384], bf16, tag="wqkvT")
            wgT = big.tile([128, CPD * 128], bf16, tag="wgT")
            woT = small.tile([128, D], f32r, tag="woT")
            cvec = small.tile([65, 1152], f32r, tag="cvec")
            cmat = small.tile([128, 129], f32r, tag="cmat")
            cgb = small.tile([128, 1], f32, tag="cgb")
            zcol = small.tile([128, 1], f32, tag="zcol")
            nc.vector.memset(zcol, 0.0)
            onescb = small.tile([128, 1], bf16, tag="onescb")
            nc.vector.memset(onescb, 1.0)
            # stats slab (65, 3N) f32r: column slabs at bases 0/32/64 chosen
            # so every multi-input DVE op sees equal input bases.
            stats = small.tile([65, 3 * N], f32r, tag="stats")
            epsc = small.tile([65, 1], f32, tag="epsc")
            nc.vector.memset(epsc, EPS)
            csqkv = cvec[0:1, 0:384]
            csg = cvec[0:1, 384:512]
            sel_a = cvec[32:33, 512:640]
            sel_b = cvec[32:33, 640:768]
            cbq = cvec[0:1, 768:1152]
            onesr = cvec[64:65, 0:128]
            ident = cmat[:, 0:128]
            onesc = cmat[:, 128:129]
            s1 = stats[0:1, 0:N]                  # @p0
            s2 = stats[32:33, 0:N]                # @p32
            t1 = stats[32:33, N:2 * N]            # @p32
            t2 = stats[64:65, 0:N]                # @p64; becomes rstd in place
            rstd = t2
            sqv = stats[0:1, N:2 * N]             # @p0 (pairs cbq)
            dens = stats[32:33, 0:2 * N]          # aliases s2/t1 (dead) @p32
            dpw = stats[32:33, 2 * N:3 * N]       # @p32 (pairs sels)

            # ---------------- load DMAs (dependency-first order) ----------------
            nc.sync.dma_start(out=cmat, in_=cmat_d.ap())
            nc.sync.dma_start(out=cvec, in_=cvec_d.ap())
            nc.sync.dma_start(out=cgb, in_=cgb_d.ap())
            xTv = xT_d.ap().rearrange("(c p) q -> c p q", p=128)
            wv = wqkvT_d.ap().rearrange("(c p) m -> c p m", p=128)
            gv = wgT_d.ap().rearrange("(c p) m -> c p m", p=128)
            for i in range(CPD):
                nc.sync.dma_start(out=xT[:, i * N:(i + 1) * N], in_=xTv[i])
                nc.sync.dma_start(out=wqkvT[:, i * 384:(i + 1) * 384], in_=wv[i])
                nc.sync.dma_start(out=wgT[:, i * 128:(i + 1) * 128], in_=gv[i])
            nc.sync.dma_start(out=woT, in_=woT_d.ap())

            # ---------------- phase A: stats ----------------
            ctx_ps = tc.tile_pool(name="ps", bufs=1, space="PSUM")
            PSP = ctx_ps.__enter__()
            with tc.high_priority():
                # per-qc chains so rstd[qc] unblocks as soon as its stats land
                for qc in range(NQ):
                    q0, q1 = qc * QB, (qc + 1) * QB
                    ps = PSP.tile([1, QB], f32, tag="w1", bufs=4)
                    for i in range(CPD):
                        nc.tensor.matmul(
                            ps, onescb, xT[:, i * N + q0:i * N + q1],
                            start=(i == 0), stop=(i == CPD - 1))
                    nc.scalar.copy(out=s1[:, q0:q1], in_=ps)
                    ps2 = PSP.tile([1, QB], f32, tag="w1", bufs=4)
                    for i in range(CPD):
                        xs = xsqp.tile([128, QB], bf16, tag="xsq")
                        sl = xT[:, i * N + q0:i * N + q1]
                        if i % 2 == 0:
                            nc.vector.tensor_mul(xs, sl, sl)
                        else:
                            nc.scalar.square(xs, sl)
                        nc.tensor.matmul(ps2, onescb, xs,
                                         start=(i == 0), stop=(i == CPD - 1))
                    nc.scalar.copy(out=s2[:, q0:q1], in_=ps2)
                # one wide op per chain stage (avoids per-qc ping-pong)
                nc.vector.tensor_mul(t1, s1, s1)
                nc.vector.scalar_tensor_tensor(
                    t2, t1, -1.0 / D, s2, op0=ALU.mult, op1=ALU.add)
                if has_cb:
                    tvs = work.tile([65, N], f32r, tag="tv")
                    tv = tvs[64:65, :]
                    nc.vector.tensor_scalar(
                        tv, t2, 1.0 / D, EPS, op0=ALU.mult, op1=ALU.add)
                # rstd = 1/sqrt(var+eps) in one ACT op (one table set)
                nc.scalar.activation(rstd, t2, AF.Abs_reciprocal_sqrt,
                                     bias=epsc[64:65], scale=1.0 / D)
                if has_cb:
                    nc.vector.tensor_mul(sqv, tv, rstd)
                for qc in range(NQ):
                    q0, q1 = qc * QB, (qc + 1) * QB
                    pb = PSP.tile([128, QB], f32, tag="w1", bufs=4)
                    nc.tensor.matmul(pb, onesr, rstd[:, q0:q1],
                                     start=True, stop=True)
                    nc.vector.tensor_copy(rstdb[:, q0:q1], pb)

            # ---------------- phase B: QKV + gate ----------------
            if True:
                def vtrans_now():
                    for h in range(HPC):
                        for kc in range(KC):
                            pv = PSP.tile([128, 64], f32r, tag="w1", bufs=4)
                            nc.tensor.transpose(
                                pv,
                                vT[h * 64:(h + 1) * 64,
                                   kc * 128:(kc + 1) * 128],
                                ident[h * 64:(h + 1) * 64,
                                      h * 64:(h + 1) * 64])
                            base = (h * KC + kc) * 65
                            nc.vector.tensor_copy(vaug[:, base:base + 64],
                                                  pv.bitcast(f32))
                            nc.vector.memset(vaug[:, base + 64:base + 65], 1.0)
                dests = (vT, qT, kT)
                kq_lim = (KC * 128 + QB - 1) // QB   # k/v cols needed
                for j in (1, 2, 0):         # q, k first (scores), v last
                    for qc in range(NQ):
                        if j != 1 and qc >= kq_lim:
                            continue        # masked-out key columns
                        ps = PSP.tile([128, QB], f32, tag="w1", bufs=4)
                        for i in range(CPD):
                            nc.tensor.matmul(
                                ps,
                                wqkvT[:, i * 384 + j * 128:i * 384 + (j + 1) * 128],
                                xT[:, i * N + qc * QB:i * N + (qc + 1) * QB],
                                start=(i == 0), stop=False)
                        nc.tensor.matmul(
                            ps, csqkv[:, j * 128:(j + 1) * 128],
                            s1[:, qc * QB:(qc + 1) * QB],
                            start=False, stop=(not has_cb))
                        if has_cb:
                            nc.tensor.matmul(
                                ps, cbq[:, j * 128:(j + 1) * 128],
                                sqv[:, qc * QB:(qc + 1) * QB],
                                start=False, stop=True)
                        nc.vector.tensor_mul(
                            dests[j][:, qc * QB:(qc + 1) * QB], ps,
                            rstdb[:, qc * QB:(qc + 1) * QB])
                    if j == 0:
                        vtrans_now()
                # (vtrans emitted inside j==0 via vtrans_now)
                for h in []:
                    for kc in range(KC):
                        pv = PSP.tile([128, 64], f32r, tag="w1", bufs=4)
                        nc.tensor.transpose(
                            pv,
                            vT[h * 64:(h + 1) * 64, kc * 128:(kc + 1) * 128],
                            ident[h * 64:(h + 1) * 64, h * 64:(h + 1) * 64])
                        base = (h * KC + kc) * 65
                        nc.vector.tensor_copy(vaug[:, base:base + 64],
                                              pv.bitcast(f32))
                        nc.vector.memset(vaug[:, base + 64:base + 65], 1.0)

            # ---------------- gate (before attention; PE filler) --------
            def emit_gate():
                # gate matmuls act as PE filler while ACT grinds exp
                if True:
                    for qc in range(NQ):
                        ps = PSP.tile([128, QB], f32, tag="w1", bufs=4)
                        for i in range(CPD):
                            nc.tensor.matmul(
                                ps, wgT[:, i * 128:(i + 1) * 128],
                                xT[:, i * N + qc * QB:i * N + (qc + 1) * QB],
                                start=(i == 0), stop=False)
                        nc.tensor.matmul(
                            ps, csg, s1[:, qc * QB:(qc + 1) * QB],
                            start=False, stop=True)
                        gs = gT[:, qc * QB:(qc + 1) * QB]
                        nc.vector.tensor_mul(
                            gs, ps, rstdb[:, qc * QB:(qc + 1) * QB])
                        # sigmoid(z) = 0.5 + 0.5*tanh(z/2); tanh shares the
                        # exp table set so ACT never switches sets mid-kernel
                        nc.scalar.activation(gs, gs, AF.Tanh, bias=cgb,
                                             scale=0.5)
                        nc.vector.tensor_scalar(
                            gs, gs, 0.5, 0.5,
                            op0=ALU.mult, op1=ALU.add)

            # ---------------- phase C: attention ----------------
            for h in range(HPC):
                if True:
                    avs = []
                    for _qc in range(NQ):
                        av_t = PSP.tile([65, QB], f32, tag="w1", bufs=4)
                        avs.append(av_t)
                    for kc in range(KC):
                        eb = ebp.tile([128, N], bf16, tag="expb")
                        nc.sync.dma_start(
                            out=eb, in_=expb_d.ap()[h, kc * 128:(kc + 1) * 128, :])
                        pT = pTp.tile([128, N], bf16, tag="pT")
                        for half in range(2):
                            sps = PSP.tile([128, 2 * QB], f32, tag="w2", bufs=2)
                            for qq in range(2):
                                qc = half * 2 + qq
                                nc.tensor.matmul(
                                    sps[:, qq * QB:(qq + 1) * QB],
                                    kT[h * 64:(h + 1) * 64, kc * 128:(kc + 1) * 128],
                                    qT[h * 64:(h + 1) * 64, qc * QB:(qc + 1) * QB],
                                    start=True, stop=True)
                            pe = pT[:, half * 2 * QB:(half + 1) * 2 * QB]
                            nc.scalar.activation(
                                pe, sps, AF.Exp, bias=zcol, scale=1.0)
                            nc.vector.tensor_mul(
                                pe, pe, eb[:, half * 2 * QB:(half + 1) * 2 * QB])
                        vbase = (h * KC + kc) * 65
                        for qc in range(NQ):
                            nc.tensor.matmul(
                                avs[qc], vaug[:, vbase:vbase + 65],
                                pT[:, qc * QB:(qc + 1) * QB],
                                start=(kc == 0), stop=(kc == KC - 1))
                    for qc in range(NQ):
                        nc.vector.tensor_copy(
                            yT[h * 64:(h + 1) * 64, qc * QB:(qc + 1) * QB],
                            avs[qc][0:64, :])
                        nc.vector.tensor_copy(
                            dens[:, h * N + qc * QB:h * N + (qc + 1) * QB],
                            avs[qc][64:65, :])
                if h == 0:
                    emit_gate()


            # ---------------- phase D: gy + out projection ----------------
            with tc.tile_pool(name="psD", bufs=2, space="PSUM") as psD, \
                 tc.tile_pool(name="psDo", bufs=2, space="PSUM") as psDo:
                for qc in range(NQ):
                    dp0 = dpw[:, 0:QB]
                    dp1 = dpw[:, QB:2 * QB]
                    nc.vector.reciprocal(
                        dp0, dens[:, 0 * N + qc * QB:0 * N + (qc + 1) * QB])
                    nc.vector.reciprocal(
                        dp1, dens[:, 1 * N + qc * QB:1 * N + (qc + 1) * QB])
                    pi = PSP.tile([128, QB], f32, tag="w1", bufs=4)
                    nc.tensor.matmul(pi, sel_a, dp0, start=True, stop=False)
                    nc.tensor.matmul(pi, sel_b, dp1, start=False, stop=True)
                    gs = gT[:, qc * QB:(qc + 1) * QB]
                    nc.vector.tensor_mul(gs, gs, pi.bitcast(f32r))
                    nc.vector.tensor_mul(gs, gs, yT[:, qc * QB:(qc + 1) * QB]
                                         .bitcast(f32r))
                    for tt in range(qc * 4, qc * 4 + 4):
                        po = PSP.tile([128, D], f32, tag="w2", bufs=2)
                        for oc in range(2):
                            nc.tensor.matmul(
                                po[:, oc * QB:(oc + 1) * QB],
                                gT[:, tt * 128:(tt + 1) * 128],
                                woT[:, oc * QB:(oc + 1) * QB],
                                start=True, stop=True)
                        ot = outs.tile([128, D], bf16, tag="outsb")
                        nc.scalar.copy(out=ot[:, 0:QB], in_=po[:, 0:QB])
                        nc.vector.tensor_copy(ot[:, QB:D], po[:, QB:D])
                        nc.sync.dma_start(
                            out=out_d.ap()[tt * 128:(tt + 1) * 128, :], in_=ot)
            ctx_ps.__exit__(None, None, None)

    nc.finalize()
    return nc


def _get_nc(KC, has_cb):
    key = (KC, has_cb)
    if key not in _CACHE:
        _CACHE[key] = _build(KC, has_cb)
    return _CACHE[key]


def _run(inputs, trace=False):
    from concourse.bass_utils import run_bass_kernel_spmd

    in_maps, KC, has_cb = _host_prep(**inputs)
    nc = _get_nc(KC, has_cb)
    res = run_bass_kernel_spmd(
        nc, in_maps, core_ids=list(range(NCORES)), trace=trace)
    acc = np.zeros((N, D), np.float64)
    for i in range(NCORES):
        acc += np.asarray(res.results[i]["out"], np.float64)
    out = acc.astype(np.float32) + np.asarray(inputs["b_o"], np.float32)[None, :]
    return out.reshape(B, N, D), res


def kernel(**inputs):
    out, _ = _run(inputs, trace=False)
    return out


def kernel_traced(**inputs):
    return _run(inputs, trace=True)


# revision 49
# speedup vs baseline: 137.5377x; 1.0098x over previous
"""Trainium2 Bass kernel for gated attention (dense_transformer).

Module: LayerNorm -> fused QKV -> per-head scaled-dot-product attention with
additive bias + key mask -> sigmoid(gate) * attn_out -> output projection.

Shapes (hardcoded): B=1, N=2048, D=1024, H=16, HW=64.

Sharding: 2 heads per core across 8 cores (tensor-parallel over H).  Each core
computes a partial o_proj contribution over its 128 local channels; the host
sums the 8 partials and adds b_o.

Device-side math per core c (heads h0=2c, h1=2c+1):
  - x arrives transposed (D, N): channels on partitions; fp32 bits tagged
    float32r so the PE runs matmuls at 1 cycle/row (bf16 rate).
  - LN stats via ones-matmuls on PE: s1=sum(x), s2=sum(x^2) per token.
    rstd = rsqrt((s2 - s1^2/D)/D + eps).  Mean subtraction is folded into the
    QKV/gate matmuls as a rank-1 PSUM-accumulated correction
    (-colsum(W)/D) (x) s1; ln_w is folded into W on the host; the 1/sqrt(HW)
    q-scale is folded into W's q-rows on the host.
  - QKV^T = (W_c x)^T accumulated in PSUM, evicted with *rstd broadcast.
  - Scores computed TRANSPOSED: S^T[k,q] = sum_c kT[c,k] qT[c,q] (PE),
    P^T = exp(S^T) (ACT, PSUM->SBUF bf16) * expb (DVE bf16 2x), where
    expb[k,q] = exp(bias[h,q,k]) * mask[k] is precomputed on host in bf16
    (masked keys become exact zeros; softmax needs no max pass at these
    logit scales).
  - AV: y_un^T = [v | 1]-augmented matmul accumulating over key chunks;
    row 64 of the PSUM result is the softmax denominator.
  - gate^T = sigmoid(rstd*(W_g x - rank1) + b_g-per-partition) (ACT).
  - gy^T = gT * (1/den broadcast via tiny select matmul) * yT (DVE).
  - out_partial = gy^T.T @ WoT via PE, evicted ACT/DVE, DMA'd out.
"""

import numpy as np
import ml_dtypes

B, N, D, H, HW = 1, 2048, 1024, 16, 64
EPS = 1e-5
NCORES = 8
HPC = H // NCORES          # heads per core = 2
QB = 512                   # q free-dim block (one PSUM bank of fp32)
NQ = N // QB               # 4
CPD = D // 128             # 8 channel chunks
NT = N // 128              # 16 token tiles

_CACHE = {}


def _host_prep(x, bias, mask, ln_w, ln_b, W_qkv, W_o, b_o, W_g, b_g):
    """Build per-core input maps. Returns (in_maps, KC, has_cb)."""
    f32 = np.float32
    bf16 = ml_dtypes.bfloat16
    x = np.asarray(x, f32)
    bias = np.asarray(bias, f32)
    maskv = np.asarray(mask).reshape(B, N)[0].astype(np.int64)
    ln_w = np.asarray(ln_w, f32)
    ln_b = np.asarray(ln_b, f32)
    W_qkv = np.asarray(W_qkv, f32)
    W_g = np.asarray(W_g, f32)
    W_o = np.asarray(W_o, f32)

    valid = np.nonzero(maskv != 0)[0]
    L = int(valid[-1]) + 1 if valid.size else 128
    KC = (L + 127) // 128
    Lp = KC * 128

    xT = np.ascontiguousarray(x[0].T.astype(bf16))         # (D, N) bf16

    Wl = W_qkv * ln_w[None, :]                             # (3D, D)
    has_cb = bool(np.any(ln_b != 0.0))

    # expb for all heads: (H, Lp, N) bf16 = exp(bias[h, q, k]).T * mask[k]
    mk = (maskv[:L] != 0).astype(f32)
    expb_all = np.zeros((H, Lp, N), dtype=bf16)
    for h in range(H):
        eb = np.exp(bias[0, h, :, :L].T.astype(f32)) * mk[:, None]
        expb_all[h, :L, :] = eb.astype(bf16)

    in_maps = []
    for c in range(NCORES):
        h0, h1 = HPC * c, HPC * c + 1
        rows = []
        scale = []
        for part, s in ((128, 1.0), (0, HW ** -0.5), (64, 1.0)):
            # order: [v_h0 v_h1 | q_h0 q_h1 | k_h0 k_h1]
            for h in (h0, h1):
                rows.append(np.arange(h * 192 + part, h * 192 + part + 64))
                scale.append(np.full(64, s, f32))
        rows = np.concatenate(rows)
        scale = np.concatenate(scale)
        Wc = Wl[rows] * scale[:, None]                      # (384, D)
        wqkvT = np.ascontiguousarray(Wc.T.astype(bf16))     # (D, 384)
        csqkv = -Wc.sum(axis=1) / D                         # (384,)
        cbqkv = (W_qkv[rows] @ ln_b) * scale                # (384,)

        gsl = slice(c * 128, (c + 1) * 128)
        Wgc = (W_g * ln_w[None, :])[gsl]                    # (128, D)
        wgT = np.ascontiguousarray(Wgc.T.astype(bf16))      # (D, 128)
        csg = -Wgc.sum(axis=1) / D                          # (128,)
        cgb = ((W_g[gsl] @ ln_b + np.asarray(b_g, f32)[gsl]) / 2.0).reshape(128, 1)

        woT = np.ascontiguousarray(W_o[:, gsl].T)           # (128, D)

        # cvec (65, 1152): r0=[csqkv|csg|-|cbqkv], r32=[sel_a|sel_b], r64=onesr.
        cvec = np.zeros((65, 1152), f32)
        cvec[0, 0:384] = csqkv
        cvec[0, 384:512] = csg
        cvec[32, 512:576] = 1.0         # sel_a: gy rows 0:64  <- 1/den_h0
        cvec[32, 704:768] = 1.0         # sel_b: gy rows 64:128 <- 1/den_h1
        cvec[0, 768:1152] = cbqkv
        cvec[64, 0:128] = 1.0           # onesr (pairs rstd @ p64)
        # cmat (128, 129): [:,0:128]=identity, col 128 = ones column
        cmat = np.zeros((128, 129), f32)
        cmat[:, 0:128] = np.eye(128, dtype=f32)
        cmat[:, 128] = 1.0
        m = {
            "xT": xT,
            "wqkvT": wqkvT,
            "wgT": wgT,
            "woT": woT,
            "expb": np.ascontiguousarray(expb_all[h0:h1 + 1]),
            "cvec": cvec,
            "cmat": cmat,
            "cgb": np.ascontiguousarray(cgb),
        }
        in_maps.append(m)
    return in_maps, KC, has_cb


def _build(KC, has_cb):
    import concourse.bass as bass
    import concourse.mybir as mybir
    import concourse.tile as tile
    from concourse import bacc

    f32 = mybir.dt.float32
    f32r = mybir.dt.float32r
    bf16 = mybir.dt.bfloat16
    AF = mybir.ActivationFunctionType
    ALU = mybir.AluOpType

    nc = bacc.Bacc("TRN2", target_bir_lowering=False)

    xT_d = nc.declare_dram_parameter("xT", [D, N], bf16, False)
    wqkvT_d = nc.declare_dram_parameter("wqkvT", [D, 384], bf16, False)
    wgT_d = nc.declare_dram_parameter("wgT", [D, 128], bf16, False)
    woT_d = nc.declare_dram_parameter("woT", [128, D], f32r, False)
    expb_d = nc.declare_dram_parameter("expb", [HPC, KC * 128, N], bf16, False)
    cvec_d = nc.declare_dram_parameter("cvec", [65, 1152], f32r, False)
    cmat_d = nc.declare_dram_parameter("cmat", [128, 129], f32r, False)
    cgb_d = nc.declare_dram_parameter("cgb", [128, 1], f32, False)
    out_d = nc.declare_dram_parameter("out", [N, D], bf16, True)

    with tile.TileContext(nc) as tc:
        with (
            nc.allow_low_precision(reason="f32r rounding feeds PE-rate matmuls"),
            tc.tile_pool(name="big", bufs=1) as big,
            tc.tile_pool(name="small", bufs=1) as small,
            tc.tile_pool(name="pT", bufs=3) as pTp,
            tc.tile_pool(name="expb", bufs=4) as ebp,
            tc.tile_pool(name="outs", bufs=4) as outs,
            tc.tile_pool(name="work", bufs=2) as work,
            tc.tile_pool(name="xsqp", bufs=4) as xsqp,
        ):
            # ---------------- persistent SBUF ----------------
            xT = big.tile([128, CPD * N], bf16, tag="xT")      # 32K/part
            qT = big.tile([128, N], bf16, tag="qT")
            kT = big.tile([128, N], bf16, tag="kT")
            vT = big.tile([128, N], f32r, tag="vT")
            gT = big.tile([128, N], f32r, tag="gT")
            yT = big.tile([128, N], f32, tag="yT")
            rstdb = big.tile([128, N], f32, tag="rstdb")
            vaug = big.tile([128, HPC * KC * 65], bf16, tag="vaug")
            wqkvT = big.tile([128, CPD * 384], bf16, tag="wqkvT")
            wgT = big.tile([128, CPD * 128], bf16, tag="wgT")
            woT = small.tile([128, D], f32r, tag="woT")
            cvec = small.tile([65, 1152], f32r, tag="cvec")
            cmat = small.tile([128, 129], f32r, tag="cmat")
            cgb = small.tile([128, 1], f32, tag="cgb")
            zcol = small.tile([128, 1], f32, tag="zcol")
            nc.vector.memset(zcol, 0.0)
            onescb = small.tile([128, 1], bf16, tag="onescb")
            nc.vector.memset(onescb, 1.0)
            # stats slab (65, 3N) f32r: column slabs at bases 0/32/64 chosen
            # so every multi-input DVE op sees equal input bases.
            stats = small.tile([65, 3 * N], f32r, tag="stats")
            epsc = small.tile([65, 1], f32, tag="epsc")
            nc.vector.memset(epsc, EPS)
            csqkv = cvec[0:1, 0:384]
            csg = cvec[0:1, 384:512]
            sel_a = cvec[32:33, 512:640]
            sel_b = cvec[32:33, 640:768]
            cbq = cvec[0:1, 768:1152]
            onesr = cvec[64:65, 0:128]
            ident = cmat[:, 0:128]
            onesc = cmat[:, 128:129]
            s1 = stats[0:1, 0:N]                  # @p0
            s2 = stats[32:33, 0:N]                # @p32
            t1 = stats[32:33, N:2 * N]            # @p32
            t2 = stats[64:65, 0:N]                # @p64; becomes rstd in place
            rstd = t2
            sqv = stats[0:1, N:2 * N]             # @p0 (pairs cbq)
            dens = stats[32:33, 0:2 * N]          # aliases s2/t1 (dead) @p32
            dpw = stats[32:33, 2 * N:3 * N]       # @p32 (pairs sels)

            # ---------------- load DMAs (dependency-first order) ----------------
            nc.sync.dma_start(out=cmat, in_=cmat_d.ap())
            nc.sync.dma_start(out=cvec, in_=cvec_d.ap())
            nc.sync.dma_start(out=cgb, in_=cgb_d.ap())
            xTv = xT_d.ap().rearrange("(c p) q -> c p q", p=128)
            wv = wqkvT_d.ap().rearrange("(c p) m -> c p m", p=128)
            gv = wgT_d.ap().rearrange("(c p) m -> c p m", p=128)
            for i in range(CPD):
                nc.sync.dma_start(out=xT[:, i * N:(i + 1) * N], in_=xTv[i])
                nc.sync.dma_start(out=wqkvT[:, i * 384:(i + 1) * 384], in_=wv[i])
                nc.sync.dma_start(out=wgT[:, i * 128:(i + 1) * 128], in_=gv[i])
            nc.sync.dma_start(out=woT, in_=woT_d.ap())

            # ---------------- phase A: stats ----------------
            ctx_ps = tc.tile_pool(name="ps", bufs=1, space="PSUM")
            PSP = ctx_ps.__enter__()
            with tc.high_priority():
                # per-qc chains so rstd[qc] unblocks as soon as its stats land
                for qc in range(NQ):
                    q0, q1 = qc * QB, (qc + 1) * QB
                    ps = PSP.tile([1, QB], f32, tag="w1", bufs=4)
                    for i in range(CPD):
                        nc.tensor.matmul(
                            ps, onescb, xT[:, i * N + q0:i * N + q1],
                            start=(i == 0), stop=(i == CPD - 1))
                    nc.scalar.copy(out=s1[:, q0:q1], in_=ps)
                    ps2 = PSP.tile([1, QB], f32, tag="w1", bufs=4)
                    for i in range(CPD):
                        xs = xsqp.tile([128, QB], bf16, tag="xsq")
                        sl = xT[:, i * N + q0:i * N + q1]
                        if i % 2 == 0:
                            nc.vector.tensor_mul(xs, sl, sl)
                        else:
                            nc.scalar.square(xs, sl)
                        nc.tensor.matmul(ps2, onescb, xs,
                                         start=(i == 0), stop=(i == CPD - 1))
                    nc.scalar.copy(out=s2[:, q0:q1], in_=ps2)
                # one wide op per chain stage (avoids per-qc ping-pong)
                nc.vector.tensor_mul(t1, s1, s1)
                nc.vector.scalar_tensor_tensor(
                    t2, t1, -1.0 / D, s2, op0=ALU.mult, op1=ALU.add)
                if has_cb:
                    tvs = work.tile([65, N], f32r, tag="tv")
                    tv = tvs[64:65, :]
                    nc.vector.tensor_scalar(
                        tv, t2, 1.0 / D, EPS, op0=ALU.mult, op1=ALU.add)
                # rstd = 1/sqrt(var+eps) in one ACT op (one table set)
                nc.scalar.activation(rstd, t2, AF.Abs_reciprocal_sqrt,
                                     bias=epsc[64:65], scale=1.0 / D)
                if has_cb:
                    nc.vector.tensor_mul(sqv, tv, rstd)
                for qc in range(NQ):
                    q0, q1 = qc * QB, (qc + 1) * QB
                    pb = PSP.tile([128, QB], f32, tag="w1", bufs=4)
                    nc.tensor.matmul(pb, onesr, rstd[:, q0:q1],
                                     start=True, stop=True)
                    nc.vector.tensor_copy(rstdb[:, q0:q1], pb)

            # ---------------- phase B: QKV + gate ----------------
            if True:
                def vtrans_now():
                    for h in range(HPC):
                        for kc in range(KC):
                            pv = PSP.tile([128, 64], f32r, tag="w1", bufs=4)
                            nc.tensor.transpose(
                                pv,
                                vT[h * 64:(h + 1) * 64,
                                   kc * 128:(kc + 1) * 128],
                                ident[h * 64:(h + 1) * 64,
                                      h * 64:(h + 1) * 64])
                            base = (h * KC + kc) * 65
                            nc.vector.tensor_copy(vaug[:, base:base + 64],
                                                  pv.bitcast(f32))
                            nc.vector.memset(vaug[:, base + 64:base + 65], 1.0)
                dests = (vT, qT, kT)
                kq_lim = (KC * 128 + QB - 1) // QB   # k/v cols needed

                def emit_j(j):
                    for qc in range(NQ):
                        if j != 1 and qc >= kq_lim:
                            continue        # masked-out key columns
                        ps = PSP.tile([128, QB], f32, tag="w1", bufs=4)
                        for i in range(CPD):
                            nc.tensor.matmul(
                                ps,
                                wqkvT[:, i * 384 + j * 128:i * 384 + (j + 1) * 128],
                                xT[:, i * N + qc * QB:i * N + (qc + 1) * QB],
                                start=(i == 0), stop=False)
                        nc.tensor.matmul(
                            ps, csqkv[:, j * 128:(j + 1) * 128],
                            s1[:, qc * QB:(qc + 1) * QB],
                            start=False, stop=(not has_cb))
                        if has_cb:
                            nc.tensor.matmul(
                                ps, cbq[:, j * 128:(j + 1) * 128],
                                sqv[:, qc * QB:(qc + 1) * QB],
                                start=False, stop=True)
                        nc.vector.tensor_mul(
                            dests[j][:, qc * QB:(qc + 1) * QB], ps,
                            rstdb[:, qc * QB:(qc + 1) * QB])

                emit_j(1)
                emit_j(2)

                def emit_v():
                    emit_j(0)
                    vtrans_now()

            # ---------------- gate (before attention; PE filler) --------
            def emit_gate():
                # gate matmuls act as PE filler while ACT grinds exp
                if True:
                    for qc in range(NQ):
                        ps = PSP.tile([128, QB], f32, tag="w1", bufs=4)
                        for i in range(CPD):
                            nc.tensor.matmul(
                                ps, wgT[:, i * 128:(i + 1) * 128],
                                xT[:, i * N + qc * QB:i * N + (qc + 1) * QB],
                                start=(i == 0), stop=False)
                        nc.tensor.matmul(
                            ps, csg, s1[:, qc * QB:(qc + 1) * QB],
                            start=False, stop=True)
                        gs = gT[:, qc * QB:(qc + 1) * QB]
                        nc.vector.tensor_mul(
                            gs, ps, rstdb[:, qc * QB:(qc + 1) * QB])
                        # sigmoid(z) = 0.5 + 0.5*tanh(z/2); tanh shares the
                        # exp table set so ACT never switches sets mid-kernel
                        nc.scalar.activation(gs, gs, AF.Tanh, bias=cgb,
                                             scale=0.5)
                        nc.vector.tensor_scalar(
                            gs, gs, 0.5, 0.5,
                            op0=ALU.mult, op1=ALU.add)

            # ---------------- phase C: attention ----------------
            for h in range(HPC):
                if True:
                    avs = []
                    for _qc in range(NQ):
                        av_t = PSP.tile([65, QB], f32, tag="w1", bufs=4)
                        avs.append(av_t)
                    for kc in range(KC):
                        eb = ebp.tile([128, N], bf16, tag="expb")
                        nc.sync.dma_start(
                            out=eb, in_=expb_d.ap()[h, kc * 128:(kc + 1) * 128, :])
                        pT = pTp.tile([128, N], bf16, tag="pT")
                        for half in range(2):
                            sps = PSP.tile([128, 2 * QB], f32, tag="w2", bufs=2)
                            for qq in range(2):
                                qc = half * 2 + qq
                                nc.tensor.matmul(
                                    sps[:, qq * QB:(qq + 1) * QB],
                                    kT[h * 64:(h + 1) * 64, kc * 128:(kc + 1) * 128],
                                    qT[h * 64:(h + 1) * 64, qc * QB:(qc + 1) * QB],
                                    start=True, stop=True)
                            pe = pT[:, half * 2 * QB:(half + 1) * 2 * QB]
                            nc.scalar.activation(
                                pe, sps, AF.Exp, bias=zcol, scale=1.0)
                            nc.vector.tensor_mul(
                                pe, pe, eb[:, half * 2 * QB:(half + 1) * 2 * QB])
                        if h == 0 and kc == 0:
                            emit_v()   # v proj + transposes, after first scores
                        vbase = (h * KC + kc) * 65
                        for qc in range(NQ):
                            nc.tensor.matmul(
                                avs[qc], vaug[:, vbase:vbase + 65],
                                pT[:, qc * QB:(qc + 1) * QB],
                                start=(kc == 0), stop=(kc == KC - 1))
                    for qc in range(NQ):
                        nc.vector.tensor_copy(
                            yT[h * 64:(h + 1) * 64, qc * QB:(qc + 1) * QB],
                            avs[qc][0:64, :])
                        nc.vector.tensor_copy(
                            dens[:, h * N + qc * QB:h * N + (qc + 1) * QB],
                            avs[qc][64:65, :])
                if h == 0:
                    emit_gate()


            # ---------------- phase D: gy + out projection ----------------
            with tc.tile_pool(name="psD", bufs=2, space="PSUM") as psD, \
                 tc.tile_pool(name="psDo", bufs=2, space="PSUM") as psDo:
                for qc in range(NQ):
                    dp0 = dpw[:, 0:QB]
                    dp1 = dpw[:, QB:2 * QB]
                    nc.vector.reciprocal(
                        dp0, dens[:, 0 * N + qc * QB:0 * N + (qc + 1) * QB])
                    nc.vector.reciprocal(
                        dp1, dens[:, 1 * N + qc * QB:1 * N + (qc + 1) * QB])
                    pi = PSP.tile([128, QB], f32, tag="w1", bufs=4)
                    nc.tensor.matmul(pi, sel_a, dp0, start=True, stop=False)
                    nc.tensor.matmul(pi, sel_b, dp1, start=False, stop=True)
                    gs = gT[:, qc * QB:(qc + 1) * QB]
                    nc.vector.tensor_mul(gs, gs, pi.bitcast(f32r))
                    nc.vector.tensor_mul(gs, gs, yT[:, qc * QB:(qc + 1) * QB]
                                         .bitcast(f32r))
                    for tt in range(qc * 4, qc * 4 + 4):
                        po = PSP.tile([128, D], f32, tag="w2", bufs=2)
                        for oc in range(2):
                            nc.tensor.matmul(
                                po[:, oc * QB:(oc + 1) * QB],
                                gT[:, tt * 128:(tt + 1) * 128],
                                woT[:, oc * QB:(oc + 1) * QB],
                                start=True, stop=True)
                        ot = outs.tile([128, D], bf16, tag="outsb")
                        nc.scalar.copy(out=ot[:, 0:QB], in_=po[:, 0:QB])
                        nc.vector.tensor_copy(ot[:, QB:D], po[:, QB:D])
                        nc.sync.dma_start(
                            out=out_d.ap()[tt * 128:(tt + 1) * 128, :], in_=ot)
            ctx_ps.__exit__(None, None, None)

    nc.finalize()
    return nc


def _get_nc(KC, has_cb):
    key = (KC, has_cb)
    if key not in _CACHE:
        _CACHE[key] = _build(KC, has_cb)
    return _CACHE[key]


def _run(inputs, trace=False):
    from concourse.bass_utils import run_bass_kernel_spmd

    in_maps, KC, has_cb = _host_prep(**inputs)
    nc = _get_nc(KC, has_cb)
    res = run_bass_kernel_spmd(
        nc, in_maps, core_ids=list(range(NCORES)), trace=trace)
    acc = np.zeros((N, D), np.float64)
    for i in range(NCORES):
        acc += np.asarray(res.results[i]["out"], np.float64)
    out = acc.astype(np.float32) + np.asarray(inputs["b_o"], np.float32)[None, :]
    return out.reshape(B, N, D), res


def kernel(**inputs):
    out, _ = _run(inputs, trace=False)
    return out


def kernel_traced(**inputs):
    return _run(inputs, trace=True)
